# revision 8
# baseline (speedup 1.0000x reference)
"""Trainium2 Bass kernel for multi-head attention (dense_transformer).

Full module: qkv = x @ W_qkv + b_qkv; 16-head attention (d=64, N=4096);
out = attn @ W_proj + b_proj.

Sharding: tensor-parallel over heads - 2 heads per core on 8 cores. Each core
gets full x (pre-transposed to [C, N] fp16) plus its head slices of the
weights, computes its heads' attention and a partial projection [N, C] fp16;
the host sums the 8 partials in fp32 and adds b_proj (+ bv @ W_proj, since
softmax weights sum to 1 the V bias contributes a constant row).

Per-core dataflow (all matmul operands fp16; PSUM accumulates fp32):
  Prefix (minimal): x^T streamed into SBUF; K^T/V/Q^T only for what
     attention chunk 0 immediately needs; the rest just-in-time inside
     chunk 0 (JIT V/K 3+ key-blocks ahead of use). Later Q chunks are
     produced at the end of the previous attention chunk.
  Attention, per (q-chunk 256, key-block 128), software-pipelined 3 deep:
     two score matmuls (one per head, K=64) into separate PSUM banks of a
     [128,1024] tile (two matmuls into ONE bank fault the device); exp from
     PSUM split between ACT (exact, scale=1/8) and DVE (Schraudolph: fp16
     bits = s*SCH_A + SCH_B fused mul-add into a uint16 view) chosen per
     key-block by a static engine-load balancer - every PSUM-evicting op
     (exp, normalize, aoT copy, proj/Q/K/V evictions) is assigned to
     whichever of ACT/DVE has the lower projected cumulative busy-time, so
     neither engine becomes the pipeline's critical path.  AV uses p-slices
     [key, 128q] as the stationary operand and [V_h | 1] as moving,
     accumulating av[q, 65] per (head, q-tile) group - column 64 is the
     softmax denominator.  All 4 groups share one PSUM bank at 68-elem
     stride: only the first group may use start=True since start clears
     has_written bank-wide; later groups overwrite-where-unset.
  Deferred per-chunk epilogue (runs inside the NEXT chunk's loop to keep
     PE dense): per q-tile, per-partition reciprocal + scale into ao_nat
     fp16, PE transpose packs both heads into aoT [d=128, tok], then one
     K=128 proj matmul per (tok-tile, col-half) with aoT stationary,
     evicted fp16 and DMAd out.
"""

import numpy as np
from contextlib import ExitStack

NUM_CORES = 8
DIM = 1024
NUM_HEADS = 16
HDIM = 64
N = 4096
HPC = NUM_HEADS // NUM_CORES   # heads per core = 2
DPC = HPC * HDIM               # head dims per core = 128

# Schraudolph exp: fp16 bits = s*SCH_A + SCH_B  (s = raw score, logit = s/8)
SCH_A = 0.125 * 1024.0 * 1.4426950408889634
SCH_B = 15360.0 - 44.0

_NC_CACHE = {}


class Bal:
    """Static ACT/DVE load balancer: route each PSUM-evicting op to the
    engine with the lower projected finish time (costs in ns, measured)."""

    def __init__(self):
        self.t = {"act": 0.0, "dve": 0.0}

    def pick(self, c_act, c_dve):
        a = max(self.t["act"] + c_act, self.t["dve"])
        d = max(self.t["dve"] + c_dve, self.t["act"])
        eng = "act" if a <= d else "dve"
        self.t[eng] += c_act if eng == "act" else c_dve
        return eng

    def force(self, eng, cost):
        self.t[eng] += cost


EXP512 = (687.0, 690.0)
COPY128 = (367.0, 289.0)
EV512 = (687.0, 690.0)
QEV256 = (473.0, 423.0)
RECIP = 183.0


def build_nc(reps=1, trace_sim=False):
    if (reps, trace_sim) in _NC_CACHE:
        return _NC_CACHE[(reps, trace_sim)]

    import concourse.bass as bass
    import concourse.mybir as mybir
    import concourse.tile as tile
    from concourse import bacc
    from concourse.masks import make_identity

    f32 = mybir.dt.float32
    fp16 = mybir.dt.float16
    u16 = mybir.dt.uint16
    AF = mybir.ActivationFunctionType
    ALU = mybir.AluOpType
    ts = bass.ts

    nc = bacc.Bacc(trn_type="TRN2", target_bir_lowering=False, debug=False)
    xT = nc.dram_tensor("xT", [DIM, N], fp16, kind="ExternalInput").ap()
    wq = nc.dram_tensor("wq", [DIM, DPC], fp16, kind="ExternalInput").ap()
    wk = nc.dram_tensor("wk", [DIM, DPC], fp16, kind="ExternalInput").ap()
    wv = nc.dram_tensor("wv", [DIM, DPC], fp16, kind="ExternalInput").ap()
    wp = nc.dram_tensor("wp", [DPC, DIM], fp16, kind="ExternalInput").ap()
    bq = nc.dram_tensor("bq", [DPC, 1], f32, kind="ExternalInput").ap()
    bk = nc.dram_tensor("bk", [DPC, 1], f32, kind="ExternalInput").ap()
    ones = nc.dram_tensor("ones", [1, 1], fp16, kind="ExternalInput").ap()
    out = nc.dram_tensor("out", [N, DIM], fp16, kind="ExternalOutput").ap()

    with tile.TileContext(nc, trace_sim=trace_sim) as tc, ExitStack() as ctx:
        singles = ctx.enter_context(tc.tile_pool(name="singles", bufs=1))
        psum = ctx.enter_context(tc.tile_pool(name="ps", bufs=2, space="PSUM"))
        work = ctx.enter_context(tc.tile_pool(name="work", bufs=2))
        ppool = ctx.enter_context(tc.tile_pool(name="pp", bufs=8))
        opool = ctx.enter_context(tc.tile_pool(name="op", bufs=4))

        bal = Bal()
        pending = []  # evictions to attach right after the next exp
        exp_par = [0]

        def emit_exp(m, sv, pv):
            # strict alternation keeps both engines saturated with exps;
            # Schraudolph on DVE is accurate enough at any share (<=1.2e-2)
            eng = "act" if exp_par[0] == 0 else "dve"
            exp_par[0] ^= 1
            if eng == "act":
                nc.scalar.activation(pv, sv, AF.Exp, scale=0.125)
                bal.force("act", EXP512[0])
            else:
                nc.vector.tensor_scalar(
                    pv.bitcast(u16), sv, SCH_A, SCH_B, ALU.mult, ALU.add,
                )
                bal.force("dve", EXP512[1])
            # attach at most one pending eviction AFTER the exp on the same
            # engine: it can never head-of-line-block another exp there
            if pending:
                fn, costs = pending.pop(0)
                c = costs[0] if eng == "act" else costs[1]
                fn(eng)
                bal.force(eng, c)

        ident = singles.tile([128, 128], fp16, tag="ident")
        make_identity(nc, ident)
        # preload the exp table set so the first real exp pays no ~2.7us load
        warm = singles.tile([1, 1], f32, tag="warm")
        nc.gpsimd.memset(warm, 0.0)
        nc.scalar.activation(warm, warm, mybir.ActivationFunctionType.Exp)

        wq_sb = singles.tile([128, 8, DPC], fp16, tag="wq")
        wk_sb = singles.tile([128, 8, DPC], fp16, tag="wk")
        wv_sb = singles.tile([128, 8, DPC], fp16, tag="wv")
        nc.sync.dma_start(out=wq_sb, in_=wq.rearrange("(c p) m -> p c m", p=128))
        nc.sync.dma_start(out=wk_sb, in_=wk.rearrange("(c p) m -> p c m", p=128))
        nc.sync.dma_start(out=wv_sb, in_=wv.rearrange("(c p) m -> p c m", p=128))
        wp_sb = singles.tile([DPC, DIM], fp16, tag="wp")
        nc.sync.dma_start(out=wp_sb, in_=wp)
        bq_sb = singles.tile([DPC, 1], f32, tag="bq")
        bk_sb = singles.tile([DPC, 1], f32, tag="bk")
        nc.sync.dma_start(out=bq_sb, in_=bq)
        nc.sync.dma_start(out=bk_sb, in_=bk)

        # full x^T resident: [c-part, c-chunk, tok]
        xf = singles.tile([128, 8, N], fp16, tag="xf")
        qT = singles.tile([128, N], fp16, tag="qT")
        kT = singles.tile([128, N], fp16, tag="kT")
        aoT = singles.tile([128, N], fp16, tag="aoT")
        # V natural layout: [key-part, key-block*head, 65]; col 64 = ones
        v_nat = singles.tile([128, 32 * HPC, 65], fp16, tag="vnat")
        nc.sync.dma_start(
            out=v_nat[:, :, 64:65], in_=ones.to_broadcast((128, 32 * HPC, 1))
        )

        # q-chunks of 256: scores tile = 2 banks (bufs=3), av packs all 4
        # (head, q-tile) groups of 65 at stride 68 in ONE bank,
        # proj pool keeps 1 bank: 6+1+1 = 8 banks
        AVS = 68

        def emit_q_chunk(qi):
            acc = psum.tile([128, 256], f32, tag="pj", name="qacc", bufs=1)
            for c in range(8):
                nc.tensor.matmul(
                    acc, wq_sb[:, c, :], xf[:, c, ts(qi, 256)],
                    start=(c == 0), stop=(c == 7),
                )
            if bal.pick(*QEV256) == "act":
                nc.scalar.add(qT[:, ts(qi, 256)], acc, bq_sb)
            else:
                nc.vector.tensor_scalar_add(qT[:, ts(qi, 256)], acc, bq_sb)

        def emit_proj(t, j, tag="pj", defer=True):
            if tag == "big":
                pp = psum.tile([128, 1024], f32, tag="big", name="ppb",
                               bufs=3)[:, 0:512]
            else:
                pp = psum.tile([128, 512], f32, tag="pj", name="pp", bufs=1)
            nc.tensor.matmul(
                pp, aoT[:, ts(t, 128)], wp_sb[:, ts(j, 512)],
                start=True, stop=True,
            )
            ot = opool.tile([128, 512], fp16, tag="ot")

            def ev(eng):
                if eng == "act":
                    nc.scalar.copy(ot, pp)
                else:
                    nc.vector.tensor_copy(ot, pp)
                nc.sync.dma_start(out=out[ts(t, 128), ts(j, 512)], in_=ot)

            if defer:
                pending.append((ev, EV512))
            else:
                ev(bal.pick(*EV512))

        def emit_normalize(qi, av, defer=True):
            # per (q-tile, head) group: 1/denominator then scale into ao_nat
            # fp16; group g = h*2+qt at column offset g*AVS, col 64 = denom
            aos = [
                work.tile([128, 128], fp16, tag="ao", name="ao", bufs=6)
                for _ in range(2)
            ]
            # one strided reciprocal covers all 4 group denominators
            recip = work.tile([128, 4], f32, tag="rc", name="rc", bufs=6)
            dens = av[:, 0:4 * AVS].rearrange(
                "p (g c) -> p g c", c=AVS)[:, :, 64:65]
            nc.vector.reciprocal(recip, dens)
            bal.force("dve", RECIP)

            def mk(h, qt):
                g = h * 2 + qt

                def ev(eng):
                    if eng == "act":
                        nc.scalar.mul(
                            aos[qt][:, ts(h, 64)],
                            av[:, g * AVS:g * AVS + 64],
                            recip[:, g:g + 1],
                        )
                    else:
                        nc.vector.tensor_scalar(
                            aos[qt][:, ts(h, 64)],
                            av[:, g * AVS:g * AVS + 64],
                            recip[:, g:g + 1], None, ALU.mult,
                        )
                return ev

            for h in range(2):
                for qt in range(2):
                    if defer:
                        pending.append((mk(h, qt), COPY128))
                    else:
                        mk(h, qt)(bal.pick(*COPY128))
            return aos

        def emit_transpose(qi, qt, ao_nat, defer=True):
            tp = psum.tile([128, 128], fp16, tag="pj", name="tp", bufs=1)
            nc.tensor.transpose(tp, ao_nat, ident)

            def ev(eng):
                if eng == "act":
                    nc.scalar.copy(aoT[:, ts(qi * 2 + qt, 128)], tp)
                else:
                    nc.vector.tensor_copy(aoT[:, ts(qi * 2 + qt, 128)], tp)

            if defer:
                pending.append((ev, COPY128))
            else:
                ev(bal.pick(*COPY128))

        for _rep in range(reps):
            # ---------------- prefix: x load, K all, V all, Q chunk 0 -------
            for n in range(8):
                for c in range(8):
                    nc.sync.dma_start(
                        out=xf[:, c, ts(n, 512)], in_=xT[ts(c, 128), ts(n, 512)]
                    )

            def emit_v(kj, defer=True):
                vacc = psum.tile([128, 512], f32, tag="pj", name="vacc",
                                 bufs=1)[:, 0:128]
                for c in range(8):
                    nc.tensor.matmul(
                        vacc, xf[:, c, ts(kj, 128)], wv_sb[:, c, :],
                        start=(c == 0), stop=(c == 7),
                    )
                vslice = v_nat[:, 2 * kj:2 * kj + 2, 0:64]
                vv = vacc.rearrange("p (h d) -> p h d", h=HPC)

                def ev(eng):
                    if eng == "act":
                        nc.scalar.copy(vslice, vv)
                    else:
                        nc.vector.tensor_copy(vslice, vv)

                if defer:
                    pending.append((ev, COPY128))
                else:
                    ev(bal.pick(*COPY128))

            def emit_k(n, defer=True):
                kacc = psum.tile([128, 1024], f32, tag="big", name="kacc",
                                 bufs=3)[:, 0:512]
                for c in range(8):
                    nc.tensor.matmul(
                        kacc, wk_sb[:, c, :], xf[:, c, ts(n, 512)],
                        start=(c == 0), stop=(c == 7),
                    )
                def ev(eng):
                    if eng == "act":
                        nc.scalar.add(kT[:, ts(n, 512)], kacc, bk_sb)
                    else:
                        nc.vector.tensor_scalar_add(kT[:, ts(n, 512)], kacc,
                                                    bk_sb)

                if defer:
                    pending.append((ev, EV512))
                else:
                    ev(bal.pick(*EV512))

            emit_k(0, defer=False)
            for kj in range(4):
                emit_v(kj, defer=False)
            emit_q_chunk(0)

            # ------------- attention with deferred epilogue -------------
            prev_av = None
            for qi in range(16):
                aos = None
                last_av = prev_av
                av = None
                proj_tasks = (
                    [((qi - 1) * 2 + tl, j) for tl in range(2) for j in range(2)]
                    if qi >= 1 else []
                )
                p_tiles = {}
                for m in range(37):
                    # scores first so each exp's input is issued as early as
                    # possible; the lag-5 avs follow with ancient deps
                    if m < 32:
                        s = psum.tile([128, 1024], f32, tag="big", name="s",
                                      bufs=3)
                        nc.tensor.matmul(
                            s[:, 0:256], kT[0:64, ts(m, 128)],
                            qT[0:64, ts(qi, 256)], start=True, stop=True,
                        )
                        nc.tensor.matmul(
                            s[:, 512:768], kT[64:128, ts(m, 128)],
                            qT[64:128, ts(qi, 256)], start=True, stop=True,
                        )
                        sv = s.rearrange("p (b c) -> p b c", b=2)[:, :, 0:256]
                        p = ppool.tile([128, 512], fp16, tag="p")
                        pv = p.rearrange("p (b c) -> p b c", b=2)
                        emit_exp(m, sv, pv)
                        p_tiles[m] = p
                    if m >= 5:
                        kj = m - 5
                        p = p_tiles.pop(kj)
                        for h in range(2):
                            for qt in range(2):
                                g = h * 2 + qt
                                # start clears has_written bank-wide: only
                                # the first group may set it
                                nc.tensor.matmul(
                                    av[:, g * AVS:g * AVS + 65],
                                    p[:, h * 256 + qt * 128:
                                      h * 256 + (qt + 1) * 128],
                                    v_nat[:, 2 * kj + h, :],
                                    start=(kj == 0 and g == 0),
                                    stop=(kj == 31),
                                    skip_group_check=True,
                                )
                    if qi >= 1 and m == 0:
                        # normalize chunk qi-1 (its last av stopped at m=36
                        # of the previous loop); all 4 muls must be EMITTED
                        # before the av bank is reallocated at m==4 (WAR)
                        aos = emit_normalize(qi - 1, last_av)
                    if m == 4:
                        av = psum.tile([128, 512], f32, tag="av", name="av",
                                       bufs=1)
                        prev_av = av
                    if qi == 0 and 2 <= m <= 29:
                        # JIT V: fill chunk 0's exp-bound PE idle; V(kj)
                        # lands 5 key-blocks ahead of its AV use.  Evictions
                        # immediate: the pj bank is reallocated next step, so
                        # a deferred reader would land after the WAR point.
                        emit_v(m + 2, defer=False)
                    if qi == 0 and m in (1, 5, 9, 13, 17, 21, 25):
                        # JIT K: K-chunk n feeds score blocks 4n..4n+3,
                        # emitted 3+ key-blocks ahead of first use (same
                        # WAR rule: the big ring slot is reused at m+1)
                        emit_k(1 + (m - 1) // 4, defer=False)
                    if aos is not None and m in (7, 13):
                        tl = (m - 7) // 6
                        emit_transpose(qi - 1, tl, aos[tl])
                    if proj_tasks and m in (11, 15, 17, 21):
                        emit_proj(*proj_tasks.pop(0))
                    if qi < 15 and m == 34:
                        # end-of-chunk Q burst overlaps the normalize
                        emit_q_chunk(qi + 1)
            # tail: epilogue of the final chunk, pipelined per tok-tile
            aos = emit_normalize(15, prev_av, defer=False)
            for tl in range(2):
                emit_transpose(15, tl, aos[tl], defer=False)
            for i, (tl, j) in enumerate(
                    [(tl, j) for tl in range(2) for j in range(2)]):
                # tail: the score ring is idle, alternate its banks in to
                # double-buffer the final projections
                emit_proj(15 * 2 + tl, j, tag=("big" if i % 2 else "pj"),
                          defer=False)

    nc.compile()
    _NC_CACHE[(reps, trace_sim)] = nc
    return nc


def make_in_maps(x, W_qkv, b_qkv, W_proj):
    x2 = np.asarray(x, dtype=np.float32).reshape(N, DIM)
    xTv = np.ascontiguousarray(x2.T.astype(np.float16))
    W_qkv = np.asarray(W_qkv, dtype=np.float32)
    W16 = W_qkv.astype(np.float16)
    b_qkv = np.asarray(b_qkv, dtype=np.float32)
    Wp16 = np.asarray(W_proj, dtype=np.float32).astype(np.float16)
    maps = []
    for m in range(NUM_CORES):
        h0 = m * DPC
        maps.append({
            "xT": xTv,
            "wq": np.ascontiguousarray(W16[:, h0:h0 + DPC]),
            "wk": np.ascontiguousarray(W16[:, DIM + h0:DIM + h0 + DPC]),
            "wv": np.ascontiguousarray(W16[:, 2 * DIM + h0:2 * DIM + h0 + DPC]),
            "wp": np.ascontiguousarray(Wp16[h0:h0 + DPC, :]),
            "bq": np.ascontiguousarray(b_qkv[h0:h0 + DPC].reshape(DPC, 1)),
            "bk": np.ascontiguousarray(
                b_qkv[DIM + h0:DIM + h0 + DPC].reshape(DPC, 1)),
            "ones": np.ones((1, 1), dtype=np.float16),
        })
    return maps


def kernel(x, W_qkv, b_qkv, W_proj, b_proj, _reps=1):
    from concourse.bass_utils import run_bass_kernel_spmd

    nc = build_nc(_reps)
    maps = make_in_maps(x, W_qkv, b_qkv, W_proj)
    res = run_bass_kernel_spmd(nc, maps, list(range(NUM_CORES)))
    total = np.zeros((N, DIM), dtype=np.float32)
    for r in res.results:
        total += r["out"].astype(np.float32)
    # V bias: softmax weights sum to 1, so +bv in attention out contributes
    # the constant row bv @ W_proj; fold it into the output bias on the host.
    bv = np.asarray(b_qkv, dtype=np.float32)[2 * DIM:3 * DIM]
    b_eff = np.asarray(b_proj, dtype=np.float32) + bv @ np.asarray(
        W_proj, dtype=np.float32)
    total = total + b_eff[None, :]
    return total.reshape(1, N, DIM).astype(np.float32)
